# revision 1
# baseline (speedup 1.0000x reference)
"""2-layer LSTM decoder on 8 trn2 NeuronCores.

Strategy: tensor-parallel over hidden/gate dims (TP=8); each core owns 128 of
the 1024 hidden dims of both layers. One fused AllGather per step carries
[h0(t) | h1(t-2)]. The 2-step lag on h1 keeps layer 1 off the serial
recurrence chain: in window t the chain is only
  AG(t-1) -> mm0h(t) + mm1b(t-2) -> cell0(t)/cell1(t-2) -> AG(t),
while mm1a(t-1) (layer-1's h0-contraction) runs inside the AG flight window.

The x-dependent and static gate contributions of layer 0 are precomputed on
the host (g0x stream, bf16), so the device only does the recurrent h-parts:
  layer0: 32 MMs  (4 gates x 8 chunks of h0)
  layer1: 32 + 32 MMs (h0-part in flight / h1-part on chain)

Host-side prep (numpy): embedding gather, x/static gate precompute, weight
slicing/transposition/casting, output reassembly.
"""
import sys, os

sys.path.insert(0, "/opt/trn_rl_repo")

import numpy as np
import ml_dtypes

import concourse.bass as bass
import concourse.tile as tile
from concourse import mybir
from concourse import bass_utils
from concourse.bass_utils import run_bass_kernel_spmd

bass_utils.upload_artifacts = lambda tmpdir: f"local:{tmpdir}"

BF16 = mybir.dt.bfloat16
F32 = mybir.dt.float32
NCORES = 8
B = 64          # batch
S = int(os.environ.get("LSTM_STEPS", "256"))  # timesteps
E = 512         # embed dim
H = 1024        # hidden
HC = H // NCORES  # 128 hidden dims per core
NPBF = ml_dtypes.bfloat16

# gate permutation: original (i,f,g,o) -> ours (i,f,o,g)
GATE_PERM = [0, 1, 3, 2]
K0H = H // 128        # 8 contraction chunks for layer-0 h part
K1 = (H + H) // 128   # 16 chunks for layer 1: 0-7 contract h0, 8-15 h1


def build():
    nc = bass.Bass(num_devices=NCORES)

    w0 = nc.declare_dram_parameter("w0", [128, 4, K0H, 128], BF16, isOutput=False)
    w1 = nc.declare_dram_parameter("w1", [128, 4, K1, 128], BF16, isOutput=False)
    g0x = nc.declare_dram_parameter("g0x", [S, 128, 4, B], F32, isOutput=False)
    static1 = nc.declare_dram_parameter("static1", [128, 4, B], F32, isOutput=False)
    # ha layout: [128, 16, B] bf16 - chunks 0..7 = h0(t), 8..15 = h1(t-2)
    ha_init = nc.declare_dram_parameter("ha_init", [128, 16, B], BF16, isOutput=False)
    h1self_init = nc.declare_dram_parameter("h1self_init", [128, B], BF16, isOutput=False)
    c0_init = nc.declare_dram_parameter("c0_init", [128, B], F32, isOutput=False)
    c1_init = nc.declare_dram_parameter("c1_init", [128, B], F32, isOutput=False)
    out_ext = nc.declare_dram_parameter("out", [S, 128, B], F32, isOutput=True)
    dbg_ext = nc.declare_dram_parameter("dbg", [S, 128, B], F32, isOutput=True)

    rg = [list(range(NCORES))]

    with tile.TileContext(nc) as tc:
        with (
            tc.tile_pool(name="wpool", bufs=1) as wpool,
            tc.tile_pool(name="state", bufs=3) as state,
            tc.tile_pool(name="work", bufs=4) as work,
            tc.tile_pool(name="xin", bufs=3) as xin,
            tc.tile_pool(name="psum", bufs=2, space="PSUM") as psum,
            tc.tile_pool(name="dram", bufs=3, space="DRAM") as dram,
        ):
            # --- resident tensors ---
            w0_sb = wpool.tile([128, 4, K0H, 128], BF16)
            nc.gpsimd.dma_start(w0_sb, w0[:, :, :, :])
            w1_sb = wpool.tile([128, 4, K1, 128], BF16)
            nc.gpsimd.dma_start(w1_sb, w1[:, :, :, :])
            st1_sb = wpool.tile([128, 4, B], F32)
            nc.gpsimd.dma_start(st1_sb, static1[:, :, :])

            # --- initial state ---
            ha = state.tile([128, 16, B], BF16, tag="ha")
            nc.gpsimd.dma_start(ha, ha_init[:, :, :])
            c0 = state.tile([128, B], F32, tag="c0")
            nc.gpsimd.dma_start(c0, c0_init[:, :])
            c1 = state.tile([128, B], F32, tag="c1")
            nc.gpsimd.dma_start(c1, c1_init[:, :])

            def mm0h(ha_t):
                """Layer-0 recurrent gates: 4 gates x 8 h0 chunks."""
                ps0 = psum.tile([128, 4, B], F32, tag="ps0")
                for g in range(4):
                    for k in range(K0H):
                        nc.tensor.matmul(
                            ps0[:, g, :], w0_sb[:, g, k, :], ha_t[:, k, :],
                            start=(k == 0), stop=(k == K0H - 1),
                        )
                return ps0

            def mm1a(ha_t):
                """Layer-1 h0-contraction (chunks 0-7), closed group; the
                partial (plus static) is banked to SBUF so no PSUM group
                spans an AllGather window (cross-window groups lose their
                partials — verified on HW)."""
                ps1 = psum.tile([128, 4, B], F32, tag="ps1a")
                for g in range(4):
                    for k in range(8):
                        nc.tensor.matmul(
                            ps1[:, g, :], w1_sb[:, g, k, :], ha_t[:, k, :],
                            start=(k == 0), stop=(k == 7),
                        )
                pa = work.tile([128, 4, B], F32, tag="part1")
                nc.vector.tensor_add(pa, ps1, st1_sb)
                return pa

            def mm1b(ha_t):
                """Layer-1 h1-contraction (chunks 8-15): own closed group."""
                ps1 = psum.tile([128, 4, B], F32, tag="ps1b")
                for g in range(4):
                    for k in range(8, K1):
                        nc.tensor.matmul(
                            ps1[:, g, :], w1_sb[:, g, k, :], ha_t[:, k, :],
                            start=(k == 8), stop=(k == K1 - 1),
                        )
                return ps1

            def cell(layer, gates_ps, st_sb, c_prev, h_bf_out, h_f32_out=None,
                     ret_gates=False):
                g_sb = work.tile([128, 4, B], F32, tag=f"g{layer}")
                nc.vector.tensor_add(g_sb, gates_ps, st_sb)
                a_sb = work.tile([128, 4, B], F32, tag=f"a{layer}")
                nc.scalar.activation(
                    a_sb[:, 0:3, :], g_sb[:, 0:3, :],
                    mybir.ActivationFunctionType.Sigmoid,
                )
                nc.scalar.activation(
                    a_sb[:, 3, :], g_sb[:, 3, :],
                    mybir.ActivationFunctionType.Tanh,
                )
                t1 = work.tile([128, B], F32, tag=f"t1{layer}")
                nc.vector.tensor_mul(t1, a_sb[:, 1, :], c_prev)         # f*c
                t2 = work.tile([128, B], F32, tag=f"t2{layer}")
                nc.vector.tensor_mul(t2, a_sb[:, 0, :], a_sb[:, 3, :])  # i*g~
                c_new = state.tile([128, B], F32, tag=f"c{layer}")
                nc.vector.tensor_add(c_new, t1, t2)
                tc_t = work.tile([128, B], F32, tag=f"tc{layer}")
                nc.scalar.activation(tc_t, c_new, mybir.ActivationFunctionType.Tanh)
                nc.vector.tensor_mul(h_bf_out, a_sb[:, 2, :], tc_t)  # o*tanh(c)
                if h_f32_out is not None:
                    nc.vector.tensor_mul(h_f32_out, a_sb[:, 2, :], tc_t)
                if ret_gates:
                    return c_new, g_sb
                return c_new

            part1_pend = None  # SBUF-banked W1h0*h0(t-1) + static1
            ha_prev = None

            for t in range(S + 2):
                sigma = t - 2  # layer-1 step computed in this window

                part1_cur = part1_pend
                if t < S:
                    gx_sb = xin.tile([128, 4, B], F32, tag="gx")
                    nc.sync.dma_start(gx_sb, g0x[t, :, :, :])
                if 0 <= sigma:
                    ps1b = mm1b(ha)
                if t < S:
                    ps0 = mm0h(ha)

                h_pair = work.tile([128, 2, B], BF16, tag="hpair")
                if 0 <= sigma:
                    h1_f32 = work.tile([128, B], F32, tag="h1f32")
                    c1 = cell(1, ps1b, part1_cur, c1, h_pair[:, 1, :], h1_f32)
                    nc.sync.dma_start(out_ext[sigma, :, :], h1_f32)

                if t < S:
                    c0 = cell(0, ps0, gx_sb, c0, h_pair[:, 0, :])
                else:
                    nc.vector.memset(h_pair[:, 0, :], 0.0)

                if sigma < 0:
                    if t == 0:
                        nc.vector.memset(h_pair[:, 1, :], 0.0)
                    else:  # t == 1: own initial h1 slice rides in AG(1)
                        nc.gpsimd.dma_start(h_pair[:, 1, :], h1self_init[:, :])

                if t <= S:
                    b_in = dram.tile([2 * 128, B], BF16, tag="bin")
                    nc.sync.dma_start(
                        b_in.rearrange("(two p) b -> p two b", p=128), h_pair
                    )
                    b_out = dram.tile([2 * 128 * NCORES, B], BF16, tag="bout")
                    nc.gpsimd.collective_compute(
                        "AllGather", mybir.AluOpType.bypass, replica_groups=rg,
                        ins=[b_in.opt()], outs=[b_out.opt()],
                    )
                    ha_prev2 = ha_prev
                    ha_prev = ha
                    ha = state.tile([128, 16, B], BF16, tag="ha")
                    b_out_v = b_out.rearrange("(c two p) b -> p two c b", two=2, p=128)
                    nc.sync.dma_start(ha[:, 8:16, :], b_out_v[:, 1, :, :])
                    nc.gpsimd.dma_start(ha[:, 0:8, :], b_out_v[:, 0, :, :])

                    # flight work: layer-1 h0-part for step t-1 reads the SAME
                    # gathered state the chain MMs used (pre-AG 'ha_prev').
                    if 0 <= t - 1 < S:
                        part1_pend = mm1a(ha_prev)

    split_multi_waits(nc)
    return nc


def split_multi_waits(nc, limit=1):
    """Walrus rejects instructions with >1 sync-wait command; split extras into
    preceding wait-only drains on the same engine."""
    n = 0
    for bb in nc.main_func.blocks:
        insts = list(bb.instructions)
        out = []
        for ins in insts:
            si = ins.sync_info
            if si is not None and si.on_wait and len(si.on_wait) > limit:
                waits = list(si.on_wait)
                pre, keep = waits[:-limit], waits[-limit:]
                for j in range(0, len(pre), limit):
                    nop = mybir.InstDrain(
                        name=f"{ins.name}-wsplit{j}",
                        engine=ins.engine,
                        sync_info=mybir.SyncInfo(on_wait=pre[j:j + limit], on_update=[]),
                    )
                    nc.register_instruction(nop)
                    out.append(nop)
                    n += 1
                ins.sync_info = mybir.SyncInfo(on_wait=keep, on_update=list(si.on_update or []))
            out.append(ins)
        bb.instructions = out
    return n


def _prepare_inputs(prev_output_tokens, source_languages, encoder_outs,
                    encoder_hiddens, encoder_cells, embed,
                    W_ih_l0, W_hh_l0, b_ih_l0, b_hh_l0,
                    W_ih_l1, W_hh_l1, b_ih_l1, b_hh_l1):
    """Numpy prep: precompute layer-0 x/static gate stream, shard weights and
    states per core. Returns list of per-core input dicts."""
    f32 = np.float32
    tok = np.asarray(prev_output_tokens)
    embed = np.asarray(embed, dtype=f32)
    X = embed[tok]                      # [B, S, E]
    context = np.asarray(encoder_outs, dtype=f32)[:, -1, :]   # [B, E]
    lang = embed[np.asarray(source_languages)]                # [B, E]

    W_ih_l0 = np.asarray(W_ih_l0, dtype=f32)
    W_hh_l0 = np.asarray(W_hh_l0, dtype=f32)
    W_ih_l1 = np.asarray(W_ih_l1, dtype=f32)
    W_hh_l1 = np.asarray(W_hh_l1, dtype=f32)

    stat = (context @ W_ih_l0[:, E:2 * E].T + lang @ W_ih_l0[:, 2 * E:3 * E].T
            + np.asarray(b_ih_l0, dtype=f32) + np.asarray(b_hh_l0, dtype=f32))  # [B, 4H]
    stat1 = (np.asarray(b_ih_l1, dtype=f32) + np.asarray(b_hh_l1, dtype=f32))   # [4H]

    # Layer-0 x gate contributions for all steps, static folded in:
    # G[t, b, :] = X[b, t, :] @ W_ih_x.T + stat[b, :]
    Xf = np.ascontiguousarray(X[:, :S, :]).reshape(B * S, E)
    G = Xf @ W_ih_l0[:, 0:E].T                       # [B*S, 4H]
    G = G.reshape(B, S, 4 * H) + stat[:, None, :]
    # -> [S, H, 4, B] with gate perm
    Gp = G.reshape(B, S, 4, H)[:, :, GATE_PERM, :]   # [B, S, 4, H]
    Gp = Gp.transpose(1, 3, 2, 0)                    # [S, H, 4, B]

    Wg0h = W_hh_l0.reshape(4, H, H)[GATE_PERM]       # (i,f,o,g) [4, H, H]
    Wcat1 = np.concatenate([W_ih_l1, W_hh_l1], axis=1)           # [4H, 2H]
    Wg1 = Wcat1.reshape(4, H, 2 * H)[GATE_PERM]
    stat1g = stat1.reshape(4, H)[GATE_PERM]                      # [4, H]

    eh = np.asarray(encoder_hiddens, dtype=f32)      # [2, B, H]
    ec = np.asarray(encoder_cells, dtype=f32)
    h0t_init = np.ascontiguousarray(eh[0].T.reshape(8, 128, B).transpose(1, 0, 2)).astype(NPBF)
    h1t_init = np.ascontiguousarray(eh[1].T.reshape(8, 128, B).transpose(1, 0, 2)).astype(NPBF)
    ha_init = np.concatenate([h0t_init, h1t_init], axis=1)   # [128, 16, B]

    in_maps = []
    for c in range(NCORES):
        sl = slice(c * HC, (c + 1) * HC)
        w0c = np.ascontiguousarray(
            Wg0h[:, sl, :].reshape(4, HC, K0H, 128).transpose(3, 0, 2, 1)
        ).astype(NPBF)
        w1c = np.ascontiguousarray(
            Wg1[:, sl, :].reshape(4, HC, K1, 128).transpose(3, 0, 2, 1)
        ).astype(NPBF)
        g0xc = np.ascontiguousarray(
            Gp[:, sl, :, :].reshape(S, HC, 4, B)
        ).astype(f32)                                # [S, 128, 4, B]
        st1c = np.ascontiguousarray(
            np.broadcast_to(stat1g[:, sl, None], (4, HC, B)).transpose(1, 0, 2)
        ).astype(f32)
        h1self = np.ascontiguousarray(eh[1][:, sl].T).astype(NPBF)
        in_maps.append({
            "w0": w0c,
            "w1": w1c,
            "g0x": g0xc,
            "static1": st1c,
            "ha_init": ha_init,
            "h1self_init": h1self,
            "c0_init": np.ascontiguousarray(ec[0].T[sl]).astype(f32),
            "c1_init": np.ascontiguousarray(ec[1].T[sl]).astype(f32),
        })
    return in_maps


_CACHED = {}


def kernel(**inputs) -> np.ndarray:
    in_maps = _prepare_inputs(**inputs)
    if "nc" not in _CACHED:
        _CACHED["nc"] = build()
    nc = _CACHED["nc"]
    trace = os.environ.get("LSTM_TRACE", "0") == "1"
    try:
        res = run_bass_kernel_spmd(nc, in_maps, list(range(NCORES)), trace=trace)
    except Exception:
        # Rare transient NRT fault (seen right after profiled runs) — retry once.
        import time as _time
        _time.sleep(2.0)
        res = run_bass_kernel_spmd(nc, in_maps, list(range(NCORES)), trace=trace)
    _CACHED["last_result"] = res
    parts = [np.asarray(res.results[c]["out"]) for c in range(NCORES)]
    full = np.concatenate(parts, axis=1)          # [S, H, B]
    return np.ascontiguousarray(full.transpose(0, 2, 1)).astype(np.float32)


if __name__ == "__main__":
    pass

